# revision 1
# baseline (speedup 1.0000x reference)
"""Bahdanau additive attention, data-parallel over batch on 8 TRN2 NeuronCores.

Math (per batch row b):
    dec_proj = W @ prev[b] + b_W                       # [A]   (computed on host: tiny)
    enc_proj[s] = U @ enc[b,s] + b_U                   # [S, A]
    energy[s] = v . tanh(dec_proj + enc_proj[s])       # [S]
    w = exp(energy);  c[b] = (w @ enc[b]) / sum(w)     # [CTX]

Device strategy (per core, 8 batches):
  - enc passed as bf16.  For each 128-row s-tile:
      * natural DMA      -> [s=128, c=1024]  (rhs for the weighted-sum matmul)
      * XBAR transpose   -> [c=128 x 8, s=128] (stationary lhsT for the U-projection)
  - U-projection: psum[s,A] = sum_k trT[c_k,s].T @ UT[c_k,A], bias folded in via a
    K=1 ones-matmul with rhs = (dec_proj[b] + b_U).
  - tanh on ScalarE, v-weighting on VectorE, sum_a via ScalarE Copy+accum.
  - exp -> w (bf16); weighted sum + denominator accumulate in PSUM across the
    32 s-tiles of a batch:  c_psum[1,512]x2 += w.T @ enc_tile, den += w.T @ ones.
  - epilogue per batch: c = c_psum * (1/den), DMA out fp32.
One HBM pass of enc per layout (2 reads of the bf16 tensor total).
"""

import sys

sys.path.insert(0, "/opt/trn_rl_repo")

import numpy as np
import ml_dtypes

import concourse.bass as bass
from concourse import bacc
import concourse.mybir as mybir
import concourse.tile as tile
from concourse.bass_utils import run_bass_kernel_spmd

B, S, A, DD, CTX = 64, 4096, 256, 1024, 1024
NCORES = 8
BL = B // NCORES  # 8 batches per core
P = 128
KC = CTX // P  # 8 contraction chunks
MT = S // P    # 32 s-tiles per batch
BF16 = mybir.dt.bfloat16
F32 = mybir.dt.float32

_CACHE = {}


def _fast_bf16(x: np.ndarray) -> np.ndarray:
    """float32 -> bfloat16 with round-to-nearest-even via integer ops
    (ml_dtypes.astype is ~50x slower on GiB-scale arrays)."""
    u = np.ascontiguousarray(x, dtype=np.float32).view(np.uint32)
    r = ((u + 0x7FFF + ((u >> 16) & 1)) >> 16).astype(np.uint16)
    return r.view(ml_dtypes.bfloat16)


def _build():
    nc = bacc.Bacc()
    enc = nc.declare_dram_parameter("enc", [BL, S, CTX], BF16, isOutput=False)
    enct = nc.declare_dram_parameter("enct", [BL, CTX, S], BF16, isOutput=False)
    ut = nc.declare_dram_parameter("ut", [CTX, A], BF16, isOutput=False)
    db = nc.declare_dram_parameter("db", [BL, A], BF16, isOutput=False)
    v = nc.declare_dram_parameter("vv", [A], BF16, isOutput=False)
    out = nc.declare_dram_parameter("out", [BL, CTX], F32, isOutput=True)

    ST = 512          # s-rows per super-tile (one ~1MB DMA per layout)
    NSUB = ST // P    # 4 proj subtiles per super-tile
    NSUP = S // ST    # 8 super-tiles per batch

    with tile.TileContext(nc) as tc:
        with (
            tc.tile_pool(name="const", bufs=1) as const,
            tc.tile_pool(name="work", bufs=4) as work,
            tc.tile_pool(name="psum", bufs=3, space="PSUM") as psum,
            tc.tile_pool(name="acc", bufs=1, space="PSUM") as accp,
        ):
            # ---- constants, loaded once ----
            ut_sb = const.tile([P, KC, A], BF16)
            nc.sync.dma_start(ut_sb[:], ut.rearrange("(k p) a -> p k a", p=P))
            db_sb = const.tile([1, BL * A], BF16)
            nc.sync.dma_start(db_sb[:], db.rearrange("b a -> (b a)")[None, :])
            v_sb = const.tile([1, A], BF16)
            nc.sync.dma_start(v_sb[:], v[None, :])
            ones_row = const.tile([1, P], BF16)
            nc.vector.memset(ones_row[:], 1.0)
            ones_col = const.tile([P, 1], BF16)
            nc.vector.memset(ones_col[:], 1.0)
            zbias0 = const.tile([P, 1], F32)
            nc.vector.memset(zbias0[:], 0.0)

            # v replicated to all 128 partitions: ones_row.T @ v_sb
            vrep_ps = psum.tile([P, A], F32, tag="proj")
            nc.tensor.matmul(vrep_ps[:], ones_row[:], v_sb[:], start=True, stop=True)
            # Walrus allows only ONE sync-wait per Activation instruction, so the
            # ScalarE stream is sequenced to observe the PE and DVE clocks up
            # front: (1) the v_rep copy waits on PE, (2) the zbias_act copy
            # waits on DVE.  The steady-state Tanh then needs only its PSUM
            # (PE) wait, and Exp's bias (zbias_act) keeps its deps on the ACT
            # semaphore where they merge into a single wait.
            v_rep = const.tile([P, A], F32)
            nc.scalar.activation(v_rep[:], vrep_ps[:],
                                 mybir.ActivationFunctionType.Copy)
            zbias = const.tile([P, 1], F32)
            nc.scalar.activation(zbias[:], zbias0[:],
                                 mybir.ActivationFunctionType.Copy)

            enct3 = enct.rearrange("b (k p) s -> b p k s", p=P)
            for b in range(BL):
                c0 = accp.tile([1, 512], F32, tag="c0")
                c1 = accp.tile([1, 512], F32, tag="c1")
                den = accp.tile([1, 1], F32, tag="den")
                for t in range(NSUP):
                    s0 = t * ST
                    nat = work.tile([P, NSUB, CTX], BF16, tag="nat")
                    nc.sync.dma_start(
                        nat[:],
                        enc[b, s0:s0 + ST, :].rearrange("(o p) c -> p o c", p=P))
                    tr = work.tile([P, KC, ST], BF16, tag="tr")
                    for u in range(NSUB):
                        for k in range(KC):
                            nc.sync.dma_start_transpose(
                                tr[:, k, u * P:(u + 1) * P],
                                nat[:, u, k * P:(k + 1) * P])

                    for u in range(NSUB):
                        j = t * NSUB + u
                        # projection + bias into PSUM [s=128, A]
                        proj = psum.tile([P, A], F32, tag="proj")
                        nc.tensor.matmul(
                            proj[:], ones_row[:], db_sb[:, b * A:(b + 1) * A],
                            start=True, stop=False,
                        )
                        for k in range(KC):
                            nc.tensor.matmul(
                                proj[:], tr[:, k, u * P:(u + 1) * P],
                                ut_sb[:, k, :],
                                start=False, stop=(k == KC - 1),
                            )

                        th = work.tile([P, A], F32, tag="th")
                        nc.scalar.activation(
                            th[:], proj[:], mybir.ActivationFunctionType.Tanh,
                            bias=zbias0[:],
                        )
                        ew = work.tile([P, A], F32, tag="ew")
                        nc.vector.tensor_mul(out=ew[:], in0=th[:], in1=v_rep[:])
                        dump = work.tile([P, A], BF16, tag="dump")
                        energy = work.tile([P, 1], F32, tag="energy")
                        nc.scalar.activation(
                            dump[:], ew[:], mybir.ActivationFunctionType.Copy,
                            accum_out=energy[:],
                        )
                        wexp = work.tile([P, 1], BF16, tag="wexp")
                        nc.scalar.activation(
                            wexp[:], energy[:], mybir.ActivationFunctionType.Exp,
                            bias=zbias[:],
                        )

                        first, last = (j == 0), (j == MT - 1)
                        nc.tensor.matmul(c0[:], wexp[:], nat[:, u, 0:512],
                                         start=first, stop=last)
                        nc.tensor.matmul(c1[:], wexp[:], nat[:, u, 512:1024],
                                         start=first, stop=last)
                        nc.tensor.matmul(den[:], wexp[:], ones_col[:],
                                         start=first, stop=last)

                rec = work.tile([1, 1], F32, tag="rec")
                nc.vector.reciprocal(rec[:], den[:])
                cout = work.tile([1, CTX], F32, tag="cout")
                nc.vector.tensor_scalar_mul(cout[:, 0:512], c0[:], rec[:])
                nc.vector.tensor_scalar_mul(cout[:, 512:1024], c1[:], rec[:])
                nc.sync.dma_start(out[b][None, :], cout[:])

    if not nc.is_finalized():
        nc.finalize()
    return nc


def kernel(previous_decoder_hidden_state, encoder_final_hidden_layers,
           W, b_W, U, b_U, v):
    prev = np.asarray(previous_decoder_hidden_state, dtype=np.float32)
    enc = np.asarray(encoder_final_hidden_layers, dtype=np.float32)
    W = np.asarray(W, dtype=np.float32)
    b_W = np.asarray(b_W, dtype=np.float32)
    U = np.asarray(U, dtype=np.float32)
    b_U = np.asarray(b_U, dtype=np.float32)
    v = np.asarray(v, dtype=np.float32)

    if "nc" not in _CACHE:
        _CACHE["nc"] = _build()
    nc = _CACHE["nc"]

    # host-side prep (tiny, except the enc cast which uses a fast bit path)
    db = (prev @ W.T + b_W + b_U).astype(ml_dtypes.bfloat16)   # [B, A]
    ut = np.ascontiguousarray(U.T).astype(ml_dtypes.bfloat16)  # [CTX, A]
    enc_bf = _fast_bf16(enc)                                   # [B, S, CTX]
    enct_bf = np.ascontiguousarray(enc_bf.transpose(0, 2, 1))  # [B, CTX, S]
    v_bf = v.astype(ml_dtypes.bfloat16)

    in_maps = []
    for i in range(NCORES):
        sl = slice(i * BL, (i + 1) * BL)
        in_maps.append({
            "enc": enc_bf[sl],
            "enct": enct_bf[sl],
            "ut": ut,
            "db": db[sl],
            "vv": v_bf,
        })

    res = run_bass_kernel_spmd(nc, in_maps, list(range(NCORES)),
                               **_CACHE.get("run_kwargs", {}))
    _CACHE["last_result"] = res
    outs = [np.asarray(r["out"]) for r in res.results]
    return np.concatenate(outs, axis=0).astype(np.float32)



# revision 3
# speedup vs baseline: 5.7210x; 5.7210x over previous
"""Bahdanau additive attention, data-parallel over batch on 8 TRN2 NeuronCores.

Math (per batch row b):
    dec_proj = W @ prev[b] + b_W                       # [A]   (host: tiny)
    enc_proj[s] = U @ enc[b,s] + b_U                   # [S, A]
    energy[s] = v . tanh(dec_proj + enc_proj[s])       # [S]
    w = exp(energy);  c[b] = (w @ enc[b]) / sum(w)     # [CTX]

Device strategy (per core, 8 batches), v2 — dual-HBM-pass, zero on-chip
transposes, PE kept warm:
  - enc is staged in BOTH layouts by the host (bf16): natural [s, c] for the
    weighted sum, and transposed [c, s] for the projection.  2 HBM passes of
    1 MiB-contiguous DMAs beat any on-chip transpose path by a wide margin
    (the XBAR route serializes ~2.5 ms on the Sync queue).
  - projection in [a, s] layout: lhsT = U^T chunk [c=128, a=128] (stationary),
    rhs = encT [c=128, s=512] streaming, accumulated over 8 c-chunks in PSUM.
    dec_proj + b_U rides for free as the per-partition bias of the Tanh
    activation (out = tanh(psum + db[a])), output fp16.
  - energy directly as columns: lhsT = tanh-tile [a=128, s=128] (data as
    weights), rhs = v column [a=128, 1]  ->  psum [s=128, 1], 2 a-chunks
    accumulated.  Exp (ScalarE) -> w column [s=128, 1] bf16.
  - weighted sum: lhsT = w column, rhs = natural tile [s=128, c=512]x2 (+ ones
    for the denominator), PSUM-accumulated over the 32 s-tiles of the batch.
  - lag-2 software pipeline (proj(sc) | energy(sc-1) | wsum(sc-2)) so every
    cross-engine dependency has a full stage of slack and the PE never idles
    (HAM stays at K=8/8).
"""

import sys

sys.path.insert(0, "/opt/trn_rl_repo")

import numpy as np
import ml_dtypes

import concourse.bass as bass
from concourse import bacc
import concourse.mybir as mybir
import concourse.tile as tile
from concourse.bass_utils import run_bass_kernel_spmd

B, S, A, DD, CTX = 64, 4096, 256, 1024, 1024
NCORES = 8
BL = B // NCORES   # 8 batches per core
P = 128
KC = CTX // P      # 8 contraction chunks
ACH = A // P       # 2 a-chunks of 128
SC = S // 512      # 8 s-chunks of 512 per batch
ST4 = 512 // P     # 4 s-tiles of 128 per s-chunk
NT = S // P        # 32 s-tiles per batch
BF16 = mybir.dt.bfloat16
FP16 = mybir.dt.float16
F32 = mybir.dt.float32

_CACHE = {}


def _fast_bf16(x: np.ndarray) -> np.ndarray:
    """float32 -> bfloat16 with round-to-nearest-even via integer ops
    (ml_dtypes.astype is ~50x slower on GiB-scale arrays)."""
    u = np.ascontiguousarray(x, dtype=np.float32).view(np.uint32)
    r = ((u + 0x7FFF + ((u >> 16) & 1)) >> 16).astype(np.uint16)
    return r.view(ml_dtypes.bfloat16)


def _build():
    nc = bacc.Bacc()
    enc = nc.declare_dram_parameter("enc", [BL, S, CTX], BF16, isOutput=False)
    enct = nc.declare_dram_parameter("enct", [BL, CTX, S], BF16, isOutput=False)
    ut = nc.declare_dram_parameter("ut", [CTX, A], BF16, isOutput=False)
    db = nc.declare_dram_parameter("db", [P, BL * ACH], F32, isOutput=False)
    vv = nc.declare_dram_parameter("vv", [P, ACH], FP16, isOutput=False)
    out = nc.declare_dram_parameter("out", [BL, CTX], F32, isOutput=True)

    Tanh = mybir.ActivationFunctionType.Tanh
    Exp = mybir.ActivationFunctionType.Exp
    Copy = mybir.ActivationFunctionType.Copy

    with tile.TileContext(nc) as tc:
        with (
            tc.tile_pool(name="const", bufs=1) as const,
            tc.tile_pool(name="data", bufs=2) as data,
            tc.tile_pool(name="th", bufs=6) as thp,
            tc.tile_pool(name="wp", bufs=3) as wp,
            tc.tile_pool(name="sm", bufs=2) as sm,
            tc.tile_pool(name="ps", bufs=3, space="PSUM") as ps,
            tc.tile_pool(name="enp", bufs=2, space="PSUM") as enp,
            tc.tile_pool(name="acc", bufs=1, space="PSUM") as accp,
        ):
            # ---- constants, loaded once ----
            ut_sb = const.tile([P, KC, A], BF16)
            nc.sync.dma_start(ut_sb[:], ut.rearrange("(k p) a -> p k a", p=P))
            db_sb = const.tile([P, BL * ACH], F32)
            nc.sync.dma_start(db_sb[:], db[:, :])
            v_sb = const.tile([P, ACH], FP16)
            nc.sync.dma_start(v_sb[:], vv[:, :])
            ones_col = const.tile([P, 1], BF16)
            nc.vector.memset(ones_col[:], 1.0)
            zbias = const.tile([P, 1], F32)
            nc.vector.memset(zbias[:], 0.0)
            # ScalarE clock warmup: observe the DMA and DVE clocks up front so
            # steady-state activations only need their PE (PSUM) wait.
            scr = const.tile([P, BL * ACH], F32)
            nc.scalar.activation(scr[:], db_sb[:], Copy)
            scr2 = const.tile([P, 1], F32)
            nc.scalar.activation(scr2[:], zbias[:], Copy)

            for b in range(BL):
                et = data.tile([P, KC, S], BF16, tag="et")
                for k in range(KC):
                    nc.sync.dma_start(et[:, k, :], enct[b, k * P:(k + 1) * P, :])
                c0 = accp.tile([1, 512], F32, tag="c0")
                c1 = accp.tile([1, 512], F32, tag="c1")
                den = accp.tile([1, 1], F32, tag="den")
                nat_tiles = {}
                stage = {}  # sc -> (th0, th1) then -> w tile
                for sc in range(SC + 2):
                    # ---- stage A: projection + tanh for s-chunk sc ----
                    if sc < SC:
                        if sc % 2 == 0:
                            g = sc // 2
                            ntile = data.tile([P, 8, CTX], BF16, tag="nat")
                            nc.sync.dma_start(
                                ntile[:],
                                enc[b, g * 1024:(g + 1) * 1024, :]
                                .rearrange("(o p) c -> p o c", p=P))
                            nat_tiles[g] = ntile
                        ths = []
                        for ach in range(ACH):
                            proj = ps.tile([P, 512], F32, tag="proj")
                            for cch in range(KC):
                                nc.tensor.matmul(
                                    proj[:],
                                    ut_sb[:, cch, ach * P:(ach + 1) * P],
                                    et[:, cch, sc * 512:(sc + 1) * 512],
                                    start=(cch == 0), stop=(cch == KC - 1),
                                )
                            th = thp.tile([P, 512], FP16, tag="th")
                            idx = b * ACH + ach
                            nc.scalar.activation(th[:], proj[:], Tanh,
                                                 bias=db_sb[:, idx:idx + 1])
                            ths.append(th)
                        stage[sc] = ths
                    # ---- stage B: energy columns + exp for s-chunk sc-1 ----
                    if 1 <= sc <= SC:
                        psc = sc - 1
                        ths = stage[psc]
                        en = enp.tile([P, ST4], F32, tag="en")
                        wt = wp.tile([P, ST4], BF16, tag="w")
                        for st in range(ST4):
                            for ach in range(ACH):
                                nc.tensor.matmul(
                                    en[:, st:st + 1],
                                    ths[ach][:, st * P:(st + 1) * P],
                                    v_sb[:, ach:ach + 1],
                                    start=(ach == 0), stop=(ach == ACH - 1),
                                )
                            nc.scalar.activation(wt[:, st:st + 1],
                                                 en[:, st:st + 1], Exp,
                                                 bias=zbias[:])
                        stage[psc] = wt
                    # ---- stage C: weighted sum for s-chunk sc-2 ----
                    if sc >= 2:
                        psc = sc - 2
                        wt = stage.pop(psc)
                        for st in range(ST4):
                            j = psc * ST4 + st
                            g, jj = j // 8, j % 8
                            first, last = (j == 0), (j == NT - 1)
                            nat = nat_tiles[g]
                            nc.tensor.matmul(c0[:], wt[:, st:st + 1],
                                             nat[:, jj, 0:512],
                                             start=first, stop=last)
                            nc.tensor.matmul(c1[:], wt[:, st:st + 1],
                                             nat[:, jj, 512:1024],
                                             start=first, stop=last)
                            nc.tensor.matmul(den[:], wt[:, st:st + 1],
                                             ones_col[:],
                                             start=first, stop=last)

                rec = sm.tile([1, 1], F32, tag="rec")
                nc.vector.reciprocal(rec[:], den[:])
                cout = sm.tile([1, CTX], F32, tag="cout")
                nc.vector.tensor_scalar_mul(cout[:, 0:512], c0[:], rec[:])
                nc.vector.tensor_scalar_mul(cout[:, 512:1024], c1[:], rec[:])
                nc.sync.dma_start(out[b][None, :], cout[:])

    if not nc.is_finalized():
        nc.finalize()
    return nc


def kernel(previous_decoder_hidden_state, encoder_final_hidden_layers,
           W, b_W, U, b_U, v):
    prev = np.asarray(previous_decoder_hidden_state, dtype=np.float32)
    enc = np.asarray(encoder_final_hidden_layers, dtype=np.float32)
    W = np.asarray(W, dtype=np.float32)
    b_W = np.asarray(b_W, dtype=np.float32)
    U = np.asarray(U, dtype=np.float32)
    b_U = np.asarray(b_U, dtype=np.float32)
    v = np.asarray(v, dtype=np.float32)

    if "nc" not in _CACHE:
        _CACHE["nc"] = _build()
    nc = _CACHE["nc"]

    # host-side prep (tiny, except the enc cast which uses a fast bit path)
    db = (prev @ W.T + b_W + b_U).astype(np.float32)            # [B, A]
    db_t = db.reshape(B, ACH, P).transpose(2, 0, 1)             # [P, B, ACH]
    ut = np.ascontiguousarray(U.T).astype(ml_dtypes.bfloat16)   # [CTX, A]
    v2 = np.ascontiguousarray(v.reshape(ACH, P).T).astype(np.float16)  # [P, ACH]
    enc_bf = _fast_bf16(enc)                                    # [B, S, CTX]
    enct_bf = np.ascontiguousarray(enc_bf.transpose(0, 2, 1))   # [B, CTX, S]

    in_maps = []
    for i in range(NCORES):
        sl = slice(i * BL, (i + 1) * BL)
        in_maps.append({
            "enc": enc_bf[sl],
            "enct": enct_bf[sl],
            "ut": ut,
            "db": np.ascontiguousarray(db_t[:, sl, :]).reshape(P, BL * ACH),
            "vv": v2,
        })

    res = run_bass_kernel_spmd(nc, in_maps, list(range(NCORES)),
                               **_CACHE.get("run_kwargs", {}))
    _CACHE["last_result"] = res
    outs = [np.asarray(r["out"]) for r in res.results]
    return np.concatenate(outs, axis=0).astype(np.float32)


# revision 6
# speedup vs baseline: 6.2770x; 1.0972x over previous
"""Bahdanau additive attention, data-parallel over batch on 8 TRN2 NeuronCores.

Math (per batch row b):
    dec_proj = W @ prev[b] + b_W                       # [A]   (host: tiny)
    enc_proj[s] = U @ enc[b,s] + b_U                   # [S, A]
    energy[s] = v . tanh(dec_proj + enc_proj[s])       # [S]
    w = exp(energy);  c[b] = (w @ enc[b]) / sum(w)     # [CTX]

Device strategy (per core, 8 batches), v2 — dual-HBM-pass, zero on-chip
transposes, PE kept warm:
  - enc is staged in BOTH layouts by the host (bf16): natural [s, c] for the
    weighted sum, and transposed [c, s] for the projection.  2 HBM passes of
    1 MiB-contiguous DMAs beat any on-chip transpose path by a wide margin
    (the XBAR route serializes ~2.5 ms on the Sync queue).
  - projection in [a, s] layout: lhsT = U^T chunk [c=128, a=128] (stationary),
    rhs = encT [c=128, s=512] streaming, accumulated over 8 c-chunks in PSUM.
    dec_proj + b_U rides for free as the per-partition bias of the Tanh
    activation (out = tanh(psum + db[a])), output fp16.
  - energy directly as columns: lhsT = tanh-tile [a=128, s=128] (data as
    weights), rhs = v column [a=128, 1]  ->  psum [s=128, 1], 2 a-chunks
    accumulated.  Exp (ScalarE) -> w column [s=128, 1] bf16.
  - weighted sum: lhsT = w column, rhs = natural tile [s=128, c=512]x2 (+ ones
    for the denominator), PSUM-accumulated over the 32 s-tiles of the batch.
  - lag-2 software pipeline (proj(sc) | energy(sc-1) | wsum(sc-2)) so every
    cross-engine dependency has a full stage of slack and the PE never idles
    (HAM stays at K=8/8).
"""

import sys

sys.path.insert(0, "/opt/trn_rl_repo")

import numpy as np
import ml_dtypes

import concourse.bass as bass
from concourse import bacc
import concourse.mybir as mybir
import concourse.tile as tile
from concourse.bass_utils import run_bass_kernel_spmd

B, S, A, DD, CTX = 64, 4096, 256, 1024, 1024
NCORES = 8
BL = B // NCORES   # 8 batches per core
P = 128
KC = CTX // P      # 8 contraction chunks
ACH = A // P       # 2 a-chunks of 128
SC = S // 512      # 8 s-chunks of 512 per batch
ST4 = 512 // P     # 4 s-tiles of 128 per s-chunk
NT = S // P        # 32 s-tiles per batch
BF16 = mybir.dt.bfloat16
FP16 = mybir.dt.float16
F32 = mybir.dt.float32

_CACHE = {}


def _fast_bf16(x: np.ndarray) -> np.ndarray:
    """float32 -> bfloat16 with round-to-nearest-even via integer ops
    (ml_dtypes.astype is ~50x slower on GiB-scale arrays)."""
    u = np.ascontiguousarray(x, dtype=np.float32).view(np.uint32)
    r = ((u + 0x7FFF + ((u >> 16) & 1)) >> 16).astype(np.uint16)
    return r.view(ml_dtypes.bfloat16)


def _build():
    nc = bacc.Bacc()
    enc = nc.declare_dram_parameter("enc", [BL, S, CTX], BF16, isOutput=False)
    enct = nc.declare_dram_parameter("enct", [BL, CTX, S], BF16, isOutput=False)
    ut = nc.declare_dram_parameter("ut", [CTX, A], BF16, isOutput=False)
    db = nc.declare_dram_parameter("db", [P, BL * ACH], F32, isOutput=False)
    vv = nc.declare_dram_parameter("vv", [P, ACH], FP16, isOutput=False)
    out = nc.declare_dram_parameter("out", [BL, CTX], F32, isOutput=True)

    Tanh = mybir.ActivationFunctionType.Tanh
    Exp = mybir.ActivationFunctionType.Exp
    Copy = mybir.ActivationFunctionType.Copy

    with tile.TileContext(nc) as tc:
        with (
            tc.tile_pool(name="const", bufs=1) as const,
            tc.tile_pool(name="data", bufs=2) as data,
            tc.tile_pool(name="natp", bufs=3) as natp,
            tc.tile_pool(name="th", bufs=4) as thp,
            tc.tile_pool(name="wp", bufs=3) as wp,
            tc.tile_pool(name="sm", bufs=1) as sm,
            tc.tile_pool(name="ps", bufs=3, space="PSUM") as ps,
            tc.tile_pool(name="enp", bufs=2, space="PSUM") as enp,
            tc.tile_pool(name="acc", bufs=1, space="PSUM") as accp,
        ):
            # ---- constants, loaded once ----
            ut_sb = const.tile([P, KC, A], BF16)
            nc.sync.dma_start(ut_sb[:], ut.rearrange("(k p) a -> p k a", p=P))
            db_sb = const.tile([P, BL * ACH], F32)
            nc.sync.dma_start(db_sb[:], db[:, :])
            v_sb = const.tile([P, ACH], FP16)
            nc.sync.dma_start(v_sb[:], vv[:, :])
            ones_col = const.tile([P, 1], BF16)
            nc.vector.memset(ones_col[:], 1.0)
            zbias = const.tile([P, 1], F32)
            nc.vector.memset(zbias[:], 0.0)
            # ScalarE clock warmup: observe the DMA and DVE clocks up front so
            # steady-state activations only need their PE (PSUM) wait.
            scr = const.tile([P, BL * ACH], F32)
            nc.scalar.activation(scr[:], db_sb[:], Copy)
            scr2 = const.tile([P, 1], F32)
            nc.scalar.activation(scr2[:], zbias[:], Copy)

            def load_et(b, halves):
                """DMA the transposed-enc strips for batch b into a fresh tile.
                halves=True splits each strip in two so the first s-chunks'
                dependencies clear sooner (used for batch 0 startup)."""
                et = data.tile([P, KC, S], BF16, tag="et")
                for k in range(KC):
                    if halves:
                        h = S // 2
                        nc.sync.dma_start(et[:, k, 0:h],
                                          enct[b, k * P:(k + 1) * P, 0:h])
                        nc.sync.dma_start(et[:, k, h:S],
                                          enct[b, k * P:(k + 1) * P, h:S])
                    else:
                        nc.sync.dma_start(et[:, k, :],
                                          enct[b, k * P:(k + 1) * P, :])
                return et

            def load_nat(b, g):
                ntile = natp.tile([P, 8, CTX], BF16, tag="nat")
                nc.sync.dma_start(
                    ntile[:],
                    enc[b, g * 1024:(g + 1) * 1024, :]
                    .rearrange("(o p) c -> p o c", p=P))
                return ntile

            et = load_et(0, halves=True)
            nat_pend = {(0, 0): load_nat(0, 0)}
            et_next = None
            for b in range(BL):
                c0 = accp.tile([1, 512], F32, tag="c0")
                c1 = accp.tile([1, 512], F32, tag="c1")
                den = accp.tile([1, 1], F32, tag="den")
                nat_tiles = {g: t for (bb, g), t in nat_pend.items() if bb == b}
                nat_pend = {k: t for k, t in nat_pend.items() if k[0] != b}
                stage = {}  # sc -> (th0, th1) then -> w tile
                for sc in range(SC + 2):
                    # ---- prefetch DMAs (program-order hoisted) ----
                    if sc == 1 and b + 1 < BL:
                        et_next = load_et(b + 1, halves=False)
                    if sc in (0, 2, 4) and sc // 2 + 1 < 4:
                        g = sc // 2 + 1
                        nat_tiles[g] = load_nat(b, g)
                    if sc == 5 and b + 1 < BL:
                        nat_pend[(b + 1, 0)] = load_nat(b + 1, 0)
                    # ---- stage A: projection + tanh for s-chunk sc ----
                    if sc < SC:
                        ths = []
                        for ach in range(ACH):
                            proj = ps.tile([P, 512], F32, tag="proj")
                            for cch in range(KC):
                                nc.tensor.matmul(
                                    proj[:],
                                    ut_sb[:, cch, ach * P:(ach + 1) * P],
                                    et[:, cch, sc * 512:(sc + 1) * 512],
                                    start=(cch == 0), stop=(cch == KC - 1),
                                )
                            th = thp.tile([P, 512], FP16, tag="th")
                            idx = b * ACH + ach
                            nc.scalar.activation(th[:], proj[:], Tanh,
                                                 bias=db_sb[:, idx:idx + 1])
                            ths.append(th)
                        stage[sc] = ths
                    # ---- stage B: energy columns + exp for s-chunk sc-1 ----
                    if 1 <= sc <= SC:
                        psc = sc - 1
                        ths = stage[psc]
                        en = enp.tile([P, ST4], F32, tag="en")
                        wt = wp.tile([P, ST4], BF16, tag="w")
                        for st in range(ST4):
                            for ach in range(ACH):
                                nc.tensor.matmul(
                                    en[:, st:st + 1],
                                    ths[ach][:, st * P:(st + 1) * P],
                                    v_sb[:, ach:ach + 1],
                                    start=(ach == 0), stop=(ach == ACH - 1),
                                )
                            nc.scalar.activation(wt[:, st:st + 1],
                                                 en[:, st:st + 1], Exp,
                                                 bias=zbias[:])
                        stage[psc] = wt
                    # ---- stage C: weighted sum for s-chunk sc-2 ----
                    if sc >= 2:
                        psc = sc - 2
                        wt = stage.pop(psc)
                        for st in range(ST4):
                            j = psc * ST4 + st
                            g, jj = j // 8, j % 8
                            first, last = (j == 0), (j == NT - 1)
                            nat = nat_tiles[g]
                            nc.tensor.matmul(c0[:], wt[:, st:st + 1],
                                             nat[:, jj, 0:512],
                                             start=first, stop=last)
                            nc.tensor.matmul(c1[:], wt[:, st:st + 1],
                                             nat[:, jj, 512:1024],
                                             start=first, stop=last)
                            nc.tensor.matmul(den[:], wt[:, st:st + 1],
                                             ones_col[:],
                                             start=first, stop=last)

                rec = sm.tile([1, 1], F32, tag="rec")
                nc.vector.reciprocal(rec[:], den[:])
                cout = sm.tile([1, CTX], F32, tag="cout")
                nc.vector.tensor_scalar_mul(cout[:, 0:512], c0[:], rec[:])
                nc.vector.tensor_scalar_mul(cout[:, 512:1024], c1[:], rec[:])
                nc.sync.dma_start(out[b][None, :], cout[:])
                et = et_next

    if not nc.is_finalized():
        nc.finalize()
    return nc


def kernel(previous_decoder_hidden_state, encoder_final_hidden_layers,
           W, b_W, U, b_U, v):
    prev = np.asarray(previous_decoder_hidden_state, dtype=np.float32)
    enc = np.asarray(encoder_final_hidden_layers, dtype=np.float32)
    W = np.asarray(W, dtype=np.float32)
    b_W = np.asarray(b_W, dtype=np.float32)
    U = np.asarray(U, dtype=np.float32)
    b_U = np.asarray(b_U, dtype=np.float32)
    v = np.asarray(v, dtype=np.float32)

    if "nc" not in _CACHE:
        _CACHE["nc"] = _build()
    nc = _CACHE["nc"]

    # host-side prep (tiny, except the enc cast which uses a fast bit path)
    db = (prev @ W.T + b_W + b_U).astype(np.float32)            # [B, A]
    db_t = db.reshape(B, ACH, P).transpose(2, 0, 1)             # [P, B, ACH]
    ut = np.ascontiguousarray(U.T).astype(ml_dtypes.bfloat16)   # [CTX, A]
    v2 = np.ascontiguousarray(v.reshape(ACH, P).T).astype(np.float16)  # [P, ACH]
    enc_bf = _fast_bf16(enc)                                    # [B, S, CTX]
    enct_bf = np.ascontiguousarray(enc_bf.transpose(0, 2, 1))   # [B, CTX, S]

    in_maps = []
    for i in range(NCORES):
        sl = slice(i * BL, (i + 1) * BL)
        in_maps.append({
            "enc": enc_bf[sl],
            "enct": enct_bf[sl],
            "ut": ut,
            "db": np.ascontiguousarray(db_t[:, sl, :]).reshape(P, BL * ACH),
            "vv": v2,
        })

    res = run_bass_kernel_spmd(nc, in_maps, list(range(NCORES)),
                               **_CACHE.get("run_kwargs", {}))
    _CACHE["last_result"] = res
    outs = [np.asarray(r["out"]) for r in res.results]
    return np.concatenate(outs, axis=0).astype(np.float32)


# revision 11
# speedup vs baseline: 6.6639x; 1.0616x over previous
"""Bahdanau additive attention, data-parallel over batch on 8 TRN2 NeuronCores.

Math (per batch row b):
    dec_proj = W @ prev[b] + b_W                       # [A]   (host: tiny)
    enc_proj[s] = U @ enc[b,s] + b_U                   # [S, A]
    energy[s] = v . tanh(dec_proj + enc_proj[s])       # [S]
    w = exp(energy);  c[b] = (w @ enc[b]) / sum(w)     # [CTX]

Device strategy (per core, 8 batches), v2 — dual-HBM-pass, zero on-chip
transposes, PE kept warm:
  - enc is staged in BOTH layouts by the host (bf16): natural [s, c] for the
    weighted sum, and transposed [c, s] for the projection.  2 HBM passes of
    1 MiB-contiguous DMAs beat any on-chip transpose path by a wide margin
    (the XBAR route serializes ~2.5 ms on the Sync queue).
  - projection in [a, s] layout: lhsT = U^T chunk [c=128, a=128] (stationary),
    rhs = encT [c=128, s=512] streaming, accumulated over 8 c-chunks in PSUM.
    dec_proj + b_U rides for free as the per-partition bias of the Tanh
    activation (out = tanh(psum + db[a])), output fp16.
  - energy directly as columns: lhsT = tanh-tile [a=128, s=128] (data as
    weights), rhs = v column [a=128, 1]  ->  psum [s=128, 1], 2 a-chunks
    accumulated.  Exp (ScalarE) -> w column [s=128, 1] bf16.
  - weighted sum: lhsT = w column, rhs = natural tile [s=128, c=512]x2 (+ ones
    for the denominator), PSUM-accumulated over the 32 s-tiles of the batch.
  - lag-2 software pipeline (proj(sc) | energy(sc-1) | wsum(sc-2)) so every
    cross-engine dependency has a full stage of slack and the PE never idles
    (HAM stays at K=8/8).
"""

import sys

sys.path.insert(0, "/opt/trn_rl_repo")

import numpy as np
import ml_dtypes

import concourse.bass as bass
from concourse import bacc
import concourse.mybir as mybir
import concourse.tile as tile
from concourse.bass_utils import run_bass_kernel_spmd

B, S, A, DD, CTX = 64, 4096, 256, 1024, 1024
NCORES = 8
BL = B // NCORES   # 8 batches per core
P = 128
KC = CTX // P      # 8 contraction chunks
ACH = A // P       # 2 a-chunks of 128
SC = S // 512      # 8 s-chunks of 512 per batch
ST4 = 512 // P     # 4 s-tiles of 128 per s-chunk
NT = S // P        # 32 s-tiles per batch
BF16 = mybir.dt.bfloat16
FP16 = mybir.dt.float16
F32 = mybir.dt.float32

_CACHE = {}


def _fast_bf16(x: np.ndarray) -> np.ndarray:
    """float32 -> bfloat16 with round-to-nearest-even via integer ops
    (ml_dtypes.astype is ~50x slower on GiB-scale arrays)."""
    u = np.ascontiguousarray(x, dtype=np.float32).view(np.uint32)
    r = ((u + 0x7FFF + ((u >> 16) & 1)) >> 16).astype(np.uint16)
    return r.view(ml_dtypes.bfloat16)


def _build():
    nc = bacc.Bacc()
    enc = nc.declare_dram_parameter("enc", [BL, S, CTX], BF16, isOutput=False)
    enct = nc.declare_dram_parameter("enct", [BL, CTX, S], BF16, isOutput=False)
    ut = nc.declare_dram_parameter("ut", [CTX, A], BF16, isOutput=False)
    db = nc.declare_dram_parameter("db", [P, BL * ACH], F32, isOutput=False)
    vv = nc.declare_dram_parameter("vv", [P, ACH], FP16, isOutput=False)
    out = nc.declare_dram_parameter("out", [BL, CTX], F32, isOutput=True)

    Tanh = mybir.ActivationFunctionType.Tanh
    Exp = mybir.ActivationFunctionType.Exp
    Copy = mybir.ActivationFunctionType.Copy

    with tile.TileContext(nc) as tc:
        with (
            tc.tile_pool(name="const", bufs=1) as const,
            tc.tile_pool(name="data", bufs=2) as data,
            tc.tile_pool(name="natp", bufs=3) as natp,
            tc.tile_pool(name="th", bufs=4) as thp,
            tc.tile_pool(name="wp", bufs=3) as wp,
            tc.tile_pool(name="sm", bufs=1) as sm,
            tc.tile_pool(name="ps", bufs=3, space="PSUM") as ps,
            tc.tile_pool(name="enp", bufs=2, space="PSUM") as enp,
            tc.tile_pool(name="acc", bufs=1, space="PSUM") as accp,
        ):
            # ---- constants, loaded once ----
            ut_sb = const.tile([P, KC, A], BF16)
            nc.sync.dma_start(ut_sb[:], ut.rearrange("(k p) a -> p k a", p=P))
            db_sb = const.tile([P, BL * ACH], F32)
            nc.sync.dma_start(db_sb[:], db[:, :])
            v_sb = const.tile([P, ACH], FP16)
            nc.sync.dma_start(v_sb[:], vv[:, :])
            ones_f32 = const.tile([P, 1], F32)
            nc.vector.memset(ones_f32[:], 1.0)
            zbias = const.tile([P, 1], F32)
            nc.vector.memset(zbias[:], 0.0)
            # ScalarE clock warmup: observe the DMA and DVE clocks up front so
            # steady-state activations only need their PE (PSUM) wait.
            scr = const.tile([P, BL * ACH], F32)
            nc.scalar.activation(scr[:], db_sb[:], Copy)
            scr2 = const.tile([P, 1], F32)
            nc.scalar.activation(scr2[:], zbias[:], Copy)

            def load_et(b, split):
                """DMA the transposed-enc strips for batch b into a fresh tile.
                split>1 cuts each strip into s-segments, issued segment-major,
                so the first s-chunks' dependencies clear sooner (batch 0)."""
                et = data.tile([P, KC, S], BF16, tag="et")
                seg = S // split
                for q in range(split):
                    for k in range(KC):
                        nc.sync.dma_start(
                            et[:, k, q * seg:(q + 1) * seg],
                            enct[b, k * P:(k + 1) * P, q * seg:(q + 1) * seg])
                return et

            def load_nat(b, g):
                ntile = natp.tile([P, 8, CTX], BF16, tag="nat")
                nc.sync.dma_start(
                    ntile[:],
                    enc[b, g * 1024:(g + 1) * 1024, :]
                    .rearrange("(o p) c -> p o c", p=P))
                return ntile

            et = load_et(0, split=8)
            nat_pend = {(0, 0): load_nat(0, 0)}
            et_next = None
            for b in range(BL):
                c0 = accp.tile([1, 512], F32, tag="c0")
                c1 = accp.tile([1, 512], F32, tag="c1")
                den = accp.tile([1, 1], F32, tag="den")
                nat_tiles = {g: t for (bb, g), t in nat_pend.items() if bb == b}
                nat_pend = {k: t for k, t in nat_pend.items() if k[0] != b}
                stage = {}  # sc -> (th0, th1) then -> w tile
                for sc in range(SC + 2):
                    # ---- prefetch DMAs (program-order hoisted) ----
                    if sc == 1 and b + 1 < BL:
                        et_next = load_et(b + 1, split=1)
                    if sc in (0, 2, 4) and sc // 2 + 1 < 4:
                        g = sc // 2 + 1
                        nat_tiles[g] = load_nat(b, g)
                    if sc == 5 and b + 1 < BL:
                        nat_pend[(b + 1, 0)] = load_nat(b + 1, 0)
                    # ---- stage A: projection + tanh for s-chunk sc ----
                    if sc < SC:
                        ths = []
                        for ach in range(ACH):
                            proj = ps.tile([P, 512], F32, tag="proj")
                            for cch in range(KC):
                                nc.tensor.matmul(
                                    proj[:],
                                    ut_sb[:, cch, ach * P:(ach + 1) * P],
                                    et[:, cch, sc * 512:(sc + 1) * 512],
                                    start=(cch == 0), stop=(cch == KC - 1),
                                )
                            th = thp.tile([P, 512], FP16, tag="th")
                            idx = b * ACH + ach
                            nc.scalar.activation(th[:], proj[:], Tanh,
                                                 bias=db_sb[:, idx:idx + 1])
                            ths.append(th)
                        stage[sc] = ths
                    # ---- stage B: energy columns + exp for s-chunk sc-1 ----
                    if 1 <= sc <= SC:
                        psc = sc - 1
                        ths = stage[psc]
                        en = enp.tile([P, ST4], F32, tag="en")
                        wt = wp.tile([P, ST4], BF16, tag="w")
                        wacc = wp.tile([P, 1], F32, tag="wacc")
                        for st in range(ST4):
                            for ach in range(ACH):
                                nc.tensor.matmul(
                                    en[:, st:st + 1],
                                    ths[ach][:, st * P:(st + 1) * P],
                                    v_sb[:, ach:ach + 1],
                                    start=(ach == 0), stop=(ach == ACH - 1),
                                )
                        # one Exp over all 4 columns; accum_out gives the
                        # per-partition partial softmax denominator for free
                        nc.scalar.activation(wt[:, 0:ST4], en[:, 0:ST4], Exp,
                                             bias=zbias[:],
                                             accum_out=wacc[:])
                        nc.tensor.matmul(den[:], ones_f32[:], wacc[:],
                                         start=(psc == 0), stop=(psc == SC - 1))
                        stage[psc] = wt
                    # ---- stage C: weighted sum for s-chunk sc-2 ----
                    if sc >= 2:
                        psc = sc - 2
                        wt = stage.pop(psc)
                        for st in range(ST4):
                            j = psc * ST4 + st
                            g, jj = j // 8, j % 8
                            first, last = (j == 0), (j == NT - 1)
                            nat = nat_tiles[g]
                            nc.tensor.matmul(c0[:], wt[:, st:st + 1],
                                             nat[:, jj, 0:512],
                                             start=first, stop=last)
                            nc.tensor.matmul(c1[:], wt[:, st:st + 1],
                                             nat[:, jj, 512:1024],
                                             start=first, stop=last)

                rec = sm.tile([1, 1], F32, tag="rec")
                nc.vector.reciprocal(rec[:], den[:])
                cout = sm.tile([1, CTX], F32, tag="cout")
                nc.vector.tensor_scalar_mul(cout[:, 0:512], c0[:], rec[:])
                nc.vector.tensor_scalar_mul(cout[:, 512:1024], c1[:], rec[:])
                nc.sync.dma_start(out[b][None, :], cout[:])
                et = et_next

    if not nc.is_finalized():
        nc.finalize()
    return nc


def kernel(previous_decoder_hidden_state, encoder_final_hidden_layers,
           W, b_W, U, b_U, v):
    prev = np.asarray(previous_decoder_hidden_state, dtype=np.float32)
    enc = np.asarray(encoder_final_hidden_layers, dtype=np.float32)
    W = np.asarray(W, dtype=np.float32)
    b_W = np.asarray(b_W, dtype=np.float32)
    U = np.asarray(U, dtype=np.float32)
    b_U = np.asarray(b_U, dtype=np.float32)
    v = np.asarray(v, dtype=np.float32)

    if "nc" not in _CACHE:
        _CACHE["nc"] = _build()
    nc = _CACHE["nc"]

    # host-side prep (tiny, except the enc cast which uses a fast bit path)
    db = (prev @ W.T + b_W + b_U).astype(np.float32)            # [B, A]
    db_t = db.reshape(B, ACH, P).transpose(2, 0, 1)             # [P, B, ACH]
    ut = np.ascontiguousarray(U.T).astype(ml_dtypes.bfloat16)   # [CTX, A]
    v2 = np.ascontiguousarray(v.reshape(ACH, P).T).astype(np.float16)  # [P, ACH]
    enc_bf = _fast_bf16(enc)                                    # [B, S, CTX]
    enct_bf = np.ascontiguousarray(enc_bf.transpose(0, 2, 1))   # [B, CTX, S]

    in_maps = []
    for i in range(NCORES):
        sl = slice(i * BL, (i + 1) * BL)
        in_maps.append({
            "enc": enc_bf[sl],
            "enct": enct_bf[sl],
            "ut": ut,
            "db": np.ascontiguousarray(db_t[:, sl, :]).reshape(P, BL * ACH),
            "vv": v2,
        })

    res = run_bass_kernel_spmd(nc, in_maps, list(range(NCORES)),
                               **_CACHE.get("run_kwargs", {}))
    _CACHE["last_result"] = res
    outs = [np.asarray(r["out"]) for r in res.results]
    return np.concatenate(outs, axis=0).astype(np.float32)


# revision 16
# speedup vs baseline: 6.7898x; 1.0189x over previous
"""Bahdanau additive attention, data-parallel over batch on 8 TRN2 NeuronCores.

Math (per batch row b):
    dec_proj = W @ prev[b] + b_W                       # [A]   (host: tiny)
    enc_proj[s] = U @ enc[b,s] + b_U                   # [S, A]
    energy[s] = v . tanh(dec_proj + enc_proj[s])       # [S]
    w = exp(energy);  c[b] = (w @ enc[b]) / sum(w)     # [CTX]

Device strategy (per core, 8 batches), v2 — dual-HBM-pass, zero on-chip
transposes, PE kept warm:
  - enc is staged in BOTH layouts by the host (bf16): natural [s, c] for the
    weighted sum, and transposed [c, s] for the projection.  2 HBM passes of
    1 MiB-contiguous DMAs beat any on-chip transpose path by a wide margin
    (the XBAR route serializes ~2.5 ms on the Sync queue).
  - projection in [a, s] layout: lhsT = U^T chunk [c=128, a=128] (stationary),
    rhs = encT [c=128, s=512] streaming, accumulated over 8 c-chunks in PSUM.
    dec_proj + b_U rides for free as the per-partition bias of the Tanh
    activation (out = tanh(psum + db[a])), output fp16.
  - energy directly as columns: lhsT = tanh-tile [a=128, s=128] (data as
    weights), rhs = v column [a=128, 1]  ->  psum [s=128, 1], 2 a-chunks
    accumulated.  Exp (ScalarE) -> w column [s=128, 1] bf16.
  - weighted sum: lhsT = w column, rhs = natural tile [s=128, c=512]x2 (+ ones
    for the denominator), PSUM-accumulated over the 32 s-tiles of the batch.
  - lag-2 software pipeline (proj(sc) | energy(sc-1) | wsum(sc-2)) so every
    cross-engine dependency has a full stage of slack and the PE never idles
    (HAM stays at K=8/8).
"""

import sys

sys.path.insert(0, "/opt/trn_rl_repo")

import numpy as np
import ml_dtypes

import concourse.bass as bass
from concourse import bacc
import concourse.mybir as mybir
import concourse.tile as tile
from concourse.bass_utils import run_bass_kernel_spmd

B, S, A, DD, CTX = 64, 4096, 256, 1024, 1024
NCORES = 8
BL = B // NCORES   # 8 batches per core
P = 128
KC = CTX // P      # 8 contraction chunks
ACH = A // P       # 2 a-chunks of 128
SC = S // 512      # 8 s-chunks of 512 per batch
ST4 = 512 // P     # 4 s-tiles of 128 per s-chunk
NT = S // P        # 32 s-tiles per batch
BF16 = mybir.dt.bfloat16
FP16 = mybir.dt.float16
F32 = mybir.dt.float32

_CACHE = {}


def _fast_bf16(x: np.ndarray) -> np.ndarray:
    """float32 -> bfloat16 with round-to-nearest-even via integer ops
    (ml_dtypes.astype is ~50x slower on GiB-scale arrays)."""
    u = np.ascontiguousarray(x, dtype=np.float32).view(np.uint32)
    r = ((u + 0x7FFF + ((u >> 16) & 1)) >> 16).astype(np.uint16)
    return r.view(ml_dtypes.bfloat16)


def _build():
    nc = bacc.Bacc()
    enc = nc.declare_dram_parameter("enc", [BL, S, CTX], BF16, isOutput=False)
    enct = nc.declare_dram_parameter("enct", [BL, CTX, S], BF16, isOutput=False)
    ut = nc.declare_dram_parameter("ut", [CTX, A], BF16, isOutput=False)
    db = nc.declare_dram_parameter("db", [P, BL * ACH], F32, isOutput=False)
    vv = nc.declare_dram_parameter("vv", [P, ACH], FP16, isOutput=False)
    out = nc.declare_dram_parameter("out", [BL, CTX], F32, isOutput=True)

    Tanh = mybir.ActivationFunctionType.Tanh
    Exp = mybir.ActivationFunctionType.Exp
    Copy = mybir.ActivationFunctionType.Copy

    with tile.TileContext(nc) as tc:
        with (
            tc.tile_pool(name="const", bufs=1) as const,
            tc.tile_pool(name="data", bufs=3) as data,
            tc.tile_pool(name="natp", bufs=2) as natp,
            tc.tile_pool(name="th", bufs=4) as thp,
            tc.tile_pool(name="wp", bufs=3) as wp,
            tc.tile_pool(name="sm", bufs=1) as sm,
            tc.tile_pool(name="ps", bufs=3, space="PSUM") as ps,
            tc.tile_pool(name="enp", bufs=2, space="PSUM") as enp,
            tc.tile_pool(name="acc", bufs=1, space="PSUM") as accp,
        ):
            # ---- constants, loaded once ----
            ut_sb = const.tile([P, KC, A], BF16)
            nc.sync.dma_start(ut_sb[:], ut.rearrange("(k p) a -> p k a", p=P))
            db_sb = const.tile([P, BL * ACH], F32)
            nc.sync.dma_start(db_sb[:], db[:, :])
            v_sb = const.tile([P, ACH], FP16)
            nc.sync.dma_start(v_sb[:], vv[:, :])
            ones_f32 = const.tile([P, 1], F32)
            nc.vector.memset(ones_f32[:], 1.0)
            zbias = const.tile([P, 1], F32)
            nc.vector.memset(zbias[:], 0.0)
            # ScalarE clock warmup: observe the DMA and DVE clocks up front so
            # steady-state activations only need their PE (PSUM) wait.
            scr = const.tile([P, BL * ACH], F32)
            nc.scalar.activation(scr[:], db_sb[:], Copy)
            scr2 = const.tile([P, 1], F32)
            nc.scalar.activation(scr2[:], zbias[:], Copy)

            SH = S // 2  # 2048: s-columns per et half-tile

            def load_et(b, h, split=1):
                """DMA one half-batch of transposed enc ([c, s] layout) into a
                fresh [128, KC, 2048] tile.  split>1 cuts each strip into
                s-segments, issued segment-major, so the first s-chunks'
                dependencies clear sooner (batch 0 startup)."""
                et = data.tile([P, KC, SH], BF16, tag="et")
                seg = SH // split
                for q in range(split):
                    for k in range(KC):
                        s0 = h * SH + q * seg
                        nc.sync.dma_start(
                            et[:, k, q * seg:(q + 1) * seg],
                            enct[b, k * P:(k + 1) * P, s0:s0 + seg])
                return et

            def load_nat(b, h):
                """DMA one half-batch of natural-layout enc ([s, c]) into a
                fresh [128, 16, CTX] tile (4 MiB)."""
                ntile = natp.tile([P, 16, CTX], BF16, tag="nat")
                nc.sync.dma_start(
                    ntile[:],
                    enc[b, h * 2048:(h + 1) * 2048, :]
                    .rearrange("(o p) c -> p o c", p=P))
                return ntile

            et_tiles = {(0, 0): load_et(0, 0, split=4), (0, 1): load_et(0, 1)}
            nat_pend = {(0, 0): load_nat(0, 0)}
            for b in range(BL):
                c0 = accp.tile([1, 512], F32, tag="c0")
                c1 = accp.tile([1, 512], F32, tag="c1")
                den = accp.tile([1, 1], F32, tag="den")
                nat_tiles = {h: t for (bb, h), t in nat_pend.items() if bb == b}
                nat_pend = {k: t for k, t in nat_pend.items() if k[0] != b}
                stage = {}  # sc -> (th0, th1) then -> w tile
                for sc in range(SC + 2):
                    # ---- prefetch DMAs (program-order hoisted) ----
                    if sc == 1 and b + 1 < BL:
                        et_tiles[(b + 1, 0)] = load_et(b + 1, 0)
                    if sc == 2:
                        nat_tiles[1] = load_nat(b, 1)
                    if sc == 4 and b + 1 < BL:
                        et_tiles[(b + 1, 1)] = load_et(b + 1, 1)
                    if sc == 6 and b + 1 < BL:
                        nat_pend[(b + 1, 0)] = load_nat(b + 1, 0)
                    # ---- stage A: projection + tanh for s-chunk sc ----
                    if sc < SC:
                        ths = []
                        eth = et_tiles[(b, sc // 4)]
                        col = (sc % 4) * 512
                        for ach in range(ACH):
                            proj = ps.tile([P, 512], F32, tag="proj")
                            for cch in range(KC):
                                nc.tensor.matmul(
                                    proj[:],
                                    ut_sb[:, cch, ach * P:(ach + 1) * P],
                                    eth[:, cch, col:col + 512],
                                    start=(cch == 0), stop=(cch == KC - 1),
                                )
                            th = thp.tile([P, 512], FP16, tag="th")
                            idx = b * ACH + ach
                            nc.scalar.activation(th[:], proj[:], Tanh,
                                                 bias=db_sb[:, idx:idx + 1])
                            ths.append(th)
                        stage[sc] = ths
                    # ---- stage B: energy columns + exp for s-chunk sc-1 ----
                    if 1 <= sc <= SC:
                        psc = sc - 1
                        ths = stage[psc]
                        en = enp.tile([P, ST4], F32, tag="en")
                        wt = wp.tile([P, ST4], BF16, tag="w")
                        wacc = wp.tile([P, 1], F32, tag="wacc")
                        for st in range(ST4):
                            for ach in range(ACH):
                                nc.tensor.matmul(
                                    en[:, st:st + 1],
                                    ths[ach][:, st * P:(st + 1) * P],
                                    v_sb[:, ach:ach + 1],
                                    start=(ach == 0), stop=(ach == ACH - 1),
                                )
                        # one Exp over all 4 columns; accum_out gives the
                        # per-partition partial softmax denominator for free
                        nc.scalar.activation(wt[:, 0:ST4], en[:, 0:ST4], Exp,
                                             bias=zbias[:],
                                             accum_out=wacc[:])
                        nc.tensor.matmul(den[:], ones_f32[:], wacc[:],
                                         start=(psc == 0), stop=(psc == SC - 1))
                        stage[psc] = wt
                    # ---- stage C: weighted sum for s-chunk sc-2 ----
                    if sc >= 2:
                        psc = sc - 2
                        wt = stage.pop(psc)
                        for st in range(ST4):
                            j = psc * ST4 + st
                            h, jj = j // 16, j % 16
                            first, last = (j == 0), (j == NT - 1)
                            nat = nat_tiles[h]
                            nc.tensor.matmul(c0[:], wt[:, st:st + 1],
                                             nat[:, jj, 0:512],
                                             start=first, stop=last)
                            nc.tensor.matmul(c1[:], wt[:, st:st + 1],
                                             nat[:, jj, 512:1024],
                                             start=first, stop=last)

                rec = sm.tile([1, 1], F32, tag="rec")
                nc.vector.reciprocal(rec[:], den[:])
                cout = sm.tile([1, CTX], F32, tag="cout")
                nc.vector.tensor_scalar_mul(cout[:, 0:512], c0[:], rec[:])
                nc.vector.tensor_scalar_mul(cout[:, 512:1024], c1[:], rec[:])
                nc.sync.dma_start(out[b][None, :], cout[:])
                et_tiles.pop((b, 0), None)
                et_tiles.pop((b, 1), None)

    if not nc.is_finalized():
        nc.finalize()
    return nc


def kernel(previous_decoder_hidden_state, encoder_final_hidden_layers,
           W, b_W, U, b_U, v):
    prev = np.asarray(previous_decoder_hidden_state, dtype=np.float32)
    enc = np.asarray(encoder_final_hidden_layers, dtype=np.float32)
    W = np.asarray(W, dtype=np.float32)
    b_W = np.asarray(b_W, dtype=np.float32)
    U = np.asarray(U, dtype=np.float32)
    b_U = np.asarray(b_U, dtype=np.float32)
    v = np.asarray(v, dtype=np.float32)

    if "nc" not in _CACHE:
        _CACHE["nc"] = _build()
    nc = _CACHE["nc"]

    # host-side prep (tiny, except the enc cast which uses a fast bit path)
    db = (prev @ W.T + b_W + b_U).astype(np.float32)            # [B, A]
    db_t = db.reshape(B, ACH, P).transpose(2, 0, 1)             # [P, B, ACH]
    ut = np.ascontiguousarray(U.T).astype(ml_dtypes.bfloat16)   # [CTX, A]
    v2 = np.ascontiguousarray(v.reshape(ACH, P).T).astype(np.float16)  # [P, ACH]
    enc_bf = _fast_bf16(enc)                                    # [B, S, CTX]
    enct_bf = np.ascontiguousarray(enc_bf.transpose(0, 2, 1))   # [B, CTX, S]

    in_maps = []
    for i in range(NCORES):
        sl = slice(i * BL, (i + 1) * BL)
        in_maps.append({
            "enc": enc_bf[sl],
            "enct": enct_bf[sl],
            "ut": ut,
            "db": np.ascontiguousarray(db_t[:, sl, :]).reshape(P, BL * ACH),
            "vv": v2,
        })

    res = run_bass_kernel_spmd(nc, in_maps, list(range(NCORES)),
                               **_CACHE.get("run_kwargs", {}))
    _CACHE["last_result"] = res
    outs = [np.asarray(r["out"]) for r in res.results]
    return np.concatenate(outs, axis=0).astype(np.float32)
